# revision 21
# baseline (speedup 1.0000x reference)
"""Trainium2 Bass kernel for: out_t = silu(cumsum_t(x)) diff along T.

Reference (T, B, L, D) = (4, 2, 2048, 4096) f32:
    Y = silu(cumsum(x, axis=0)); out = concat([Y[:1], Y[1:] - Y[:-1]])

Strategy: shard L across the 8 NeuronCores (embarrassingly parallel; the
scan is over T=4 only).  Per core a raw-Bass 3-engine pipeline streams
16 chunks of [128 part x (4x1024)] fp16 through SBUF:

  SP  : ALL DMA — chunk loads plus the two output stores per chunk —
        on the SP HWDGE ring.  Every dma_start is issue-gated by a
        sequencer wait, so the ring never holds a not-ready transfer.
        The first NX loads go out as one unconditional burst (plus two
        chunk-0 slices and chunk 1 on ACT's ring, so both HWDGE units
        expand descriptors during the ramp); store(j) is then emitted
        at iteration j+NX, which makes it issue exactly when compute
        finishes chunk j — the ring stays saturated through fill,
        steady state, and drain.  First/last chunk loads are split per
        t-slice (faster ramp / shorter tail).
  DVE : running sums X1..X3 (3 fp16 tensor_adds into `at`), emitted two
        chunks ahead of the diffs so ACT never chases them, and ONE
        FD=3F tensor_sub per chunk: with Y = [Y0 Y1 Y2 Y3] contiguous
        in `yt`, d = yt[:, F:4F] - yt[:, 0:3F] computes all three
        output diffs in a single overlapping-window op.  All-16-bit
        operands keep every op in the 2x_1P perf mode (58 + FD/2
        cycles, not 151 + FD).
  ACT : pure compute — 2 silu ACTIVATEs per chunk: silu(x0) [FD=F] into
        yt[:, 0:F] and silu([X1,X2,X3]) [FD=3F] into yt[:, F:4F].

Output leaves per chunk as two stores: t0 = yt[:, 0:F] (Y0) and
t1..3 = the diff tile (both contiguous SBUF runs into the same
[NCHUNK, P, T, F] DRAM tensor).

Explicit semaphores; every dma_start carries zero attached waits (the
DMA ISA encoding only fits one) — cross-engine deps are standalone
sequencer wait_ge instructions.  Same-engine RAW chains (the running
sums) are fenced with drain-backed waits on the engine's own semaphore.

The whole pipeline is fp16: the host casts x to fp16 before upload and
widens the fp16 output back to f32 (~6.5e-4 l2 rel err, well inside the
2e-2 gate).  HBM traffic is 32 MiB per core; with the ring saturated
end-to-end the SDMA engines run it in ~84-94 us, plus ~9 us of fixed
NEFF preamble (startup barrier + engine program loads).  Measured
93-111 us run-to-run (the spread is HBM arbitration between the 8
cores: 8 x 32 MiB / 2.9 TB/s chip bandwidth = 88 us is the chip-level
floor, so a core's time depends on how fairly the early all-read phase
is arbitrated), vs ~157 us for the f32-compute/bf16-out predecessor.
Compute queues sit under the DMA roofline (ACT ~65 us, DVE ~62 us
busy); engine arithmetic internals stay f32 (DVE/ACT compute in fp32
and round on write).
"""

import sys

if "/opt/trn_rl_repo" not in sys.path:
    sys.path.insert(0, "/opt/trn_rl_repo")

import numpy as np

T, B, L, D = 4, 2, 2048, 4096
NCORES = 8
LS = L // NCORES            # 256 rows of L per core
NPOS = B * LS * D           # 2_097_152 elements per t-slice per core
P = 128                     # SBUF partitions
F = 1024                    # free-dim elements per t-slice per chunk
NCHUNK = NPOS // (P * F)    # 16 chunk iterations per core
NX = 8                      # xt (input) slot count
NA = 4                      # at (running-sum) slot count (adds run 2 ahead)
NY = 7                      # yt (silu) slot count
NO = 8                      # ob (diff) slot count
# NY/NO are deeper than strictly needed for overlap: a chunk's silu1
# waits on store_t0(i-NY) COMPLETING, and that store sits behind the
# ring's FIFO backlog — shallow slots make back-half compute ring-gated.
# Stores trail loads by NX chunks on the SP ring: store(j) is emitted at
# SP iteration j+NX, whose load slot-wait (adds(j) done) is satisfied at
# the same moment as the store's own diff(j) wait — i.e. each store is
# issued exactly when compute finishes its chunk, and the first NX loads
# go out as one unconditional burst that keeps the ring fed through the
# pipeline fill.

_NC_CACHE = {}
LAST_RESULT = None
TRACE = False
TRACE_CORES = None
TMPDIR = None


def _build_nc(use_silu: bool = True):
    import concourse.bass as bass
    from concourse import mybir

    f16 = mybir.dt.float16
    act_fn = (
        mybir.ActivationFunctionType.Silu
        if use_silu
        else mybir.ActivationFunctionType.Sigmoid
    )

    nc = bass.Bass("TRN2", debug=False)
    # Chunk-major DRAM layout [NCHUNK, P, T, F] (host repacks): each
    # partition's chunk data is one contiguous 8 KiB run, so every DMA
    # is a straight copy with maximal descriptors.
    x_d = nc.declare_dram_parameter("x", [NCHUNK, P, T, F], f16, isOutput=False)
    o_d = nc.declare_dram_parameter("out", [NCHUNK, P, T, F], f16, isOutput=True)

    TF = T * F
    # Flat free dims so every engine AP is a single contiguous run
    # (keeps the DVE perf-mode detection trivially satisfied).
    xt = [nc.alloc_sbuf_tensor(f"xt{s}", [P, TF], f16).ap() for s in range(NX)]
    at = [nc.alloc_sbuf_tensor(f"at{s}", [P, 3 * F], f16).ap() for s in range(NA)]
    yt = [nc.alloc_sbuf_tensor(f"yt{s}", [P, TF], f16).ap() for s in range(NY)]
    ob = [nc.alloc_sbuf_tensor(f"ob{s}", [P, 3 * F], f16).ap() for s in range(NO)]

    LAST = NCHUNK - 1

    # Regular-chunk load lanes: chunk 0 and LAST use dedicated split
    # per-slice sems; chunks 1..LAST-1 rotate over NX lanes.  A lane's
    # next DMA never overlaps its previous one (slot-reuse waits
    # guarantee it), so ">= 16*n" thresholds stay sound.
    lane_use = {}
    _cnt = [0] * NX
    for i in range(1, LAST):
        k = i % NX
        _cnt[k] += 1
        lane_use[i] = (k, _cnt[k])

    # Semaphore landmarks:
    #   s_add : add_t(i)  -> 3i+t  (t = 1..3)
    #   s_act : regular chunk i: silu1 -> 2i+1, silu2 -> 2i+2;
    #           LAST: silu1 -> 2L+1, then per-slice silu_t -> 2L+1+t
    #   s_diff: regular chunk i (one FD=3F sub) -> i+1;
    #           LAST per-slice d_t -> LAST + t
    # Store lanes: s_st0 (t0 stores, rotate over NY = yt slots) and
    # s_st1 (diff stores, rotate over NO = ob slots); LAST uses s_ls.

    import contextlib

    with contextlib.ExitStack() as es:
        block = es.enter_context(nc.Block())
        s_load = [es.enter_context(nc.semaphore(f"s_load{k}")) for k in range(NX)]
        s_st0 = [es.enter_context(nc.semaphore(f"s_st0_{k}")) for k in range(NY)]
        s_st1 = [es.enter_context(nc.semaphore(f"s_st1_{k}")) for k in range(NO)]
        s_add = es.enter_context(nc.semaphore("s_add"))
        s_act = es.enter_context(nc.semaphore("s_act"))
        s_diff = es.enter_context(nc.semaphore("s_diff"))
        s_l0 = [es.enter_context(nc.semaphore(f"s_l0_{t}")) for t in range(T)]
        s_ll = [es.enter_context(nc.semaphore(f"s_ll{t}")) for t in range(T)]
        s_ls = [es.enter_context(nc.semaphore(f"s_ls{t}")) for t in range(T)]

        @block.sync
        def _(sp: bass.BassEngine):
            def emit_load(i):
                j = i - NX if i >= NX else -1
                if j >= 0:
                    # xt slot free: DVE adds + ACT silu1 of chunk j done
                    # reading it.  (Transitively covers load j's
                    # completion, so this lane's previous inc is
                    # observed before re-use.)
                    sp.wait_ge(s_add, 3 * j + 3)
                    sp.wait_ge(s_act, 2 * j + 1)
                if i == 0:
                    # split: smaller first DMAs ramp the SDMA engines
                    # sooner and let compute start per slice.  Slices
                    # 2,3 (and chunk 1) are issued from ACT's ring so
                    # both HWDGE units expand descriptors concurrently
                    # during the ramp.
                    for t in range(2):
                        sp.dma_start(
                            out=xt[0][:, t * F : (t + 1) * F], in_=x_d[0][:, t]
                        ).then_inc(s_l0[t], 16)
                elif i == 1:
                    return  # issued from ACT (ramp)
                elif i == LAST:
                    for t in range(T):
                        sp.dma_start(
                            out=xt[i % NX][:, t * F : (t + 1) * F], in_=x_d[i][:, t]
                        ).then_inc(s_ll[t], 16)
                else:
                    k, _use = lane_use[i]
                    sp.dma_start(out=xt[k][:], in_=x_d[i]).then_inc(s_load[k], 16)

            def emit_store(j):
                # t0 slice (Y0) straight out of the silu tile
                sp.wait_ge(s_act, 2 * j + 1)  # silu1(j) drained
                if j >= NY:
                    # observe this lane's previous inc before re-inc'ing
                    sp.wait_ge(s_st0[j % NY], 16 * (j // NY))
                sp.dma_start(out=o_d[j][:, 0], in_=yt[j % NY][:, 0:F]).then_inc(
                    s_st0[j % NY], 16
                )
                # t1..3 diffs
                sp.wait_ge(s_diff, j + 1)
                if j >= NO:
                    sp.wait_ge(s_st1[j % NO], 16 * (j // NO))
                sp.dma_start(out=o_d[j][:, 1:4], in_=ob[j % NO][:]).then_inc(
                    s_st1[j % NO], 16
                )

            for i in range(NCHUNK):
                emit_load(i)
                if i - NX >= 0 and i - NX < LAST:
                    emit_store(i - NX)
            for j in range(max(NCHUNK - NX, 0), LAST):
                emit_store(j)
            # last chunk: per-slice stores as each slice becomes ready
            o_, y_ = ob[LAST % NO], yt[LAST % NY]
            sp.wait_ge(s_act, 2 * LAST + 1)
            sp.dma_start(out=o_d[LAST][:, 0], in_=y_[:, 0:F]).then_inc(s_ls[0], 16)
            for t in (1, 2, 3):
                sp.wait_ge(s_diff, LAST + t)
                sp.dma_start(
                    out=o_d[LAST][:, t], in_=o_[:, (t - 1) * F : t * F]
                ).then_inc(s_ls[t], 16)
            # drain: HWDGE DMAs on one ring complete in FIFO order, and
            # every store precedes the last chunk's slice stores on the
            # SP ring — waiting the four s_ls sems covers them all
            for t in range(T):
                sp.wait_ge(s_ls[t], 16)

        @block.vector
        def _(ve: bass.BassEngine):
            def emit_adds(i):
                x_, a_ = xt[i % NX], at[i % NA]
                if i >= NA:
                    # at slot free: silu2 of chunk i-NA done reading it
                    ve.wait_ge(s_act, 2 * (i - NA) + 2)
                if i == 0:
                    ve.wait_ge(s_l0[0], 16)
                    ve.wait_ge(s_l0[1], 16)
                elif i == LAST:
                    ve.wait_ge(s_ll[0], 16)
                    ve.wait_ge(s_ll[1], 16)
                else:
                    k, use = lane_use[i]
                    ve.wait_ge(s_load[k], 16 * use)
                ve.tensor_add(a_[:, 0:F], x_[:, 0:F], x_[:, F : 2 * F]).then_inc(s_add)
                # same-engine RAW needs a drain-backed sem wait
                ve.wait_ge(s_add, 3 * i + 1)
                if i == 0:
                    ve.wait_ge(s_l0[2], 16)
                elif i == LAST:
                    ve.wait_ge(s_ll[2], 16)
                ve.tensor_add(a_[:, F : 2 * F], a_[:, 0:F], x_[:, 2 * F : 3 * F]).then_inc(s_add)
                ve.wait_ge(s_add, 3 * i + 2)
                if i == 0:
                    ve.wait_ge(s_l0[3], 16)
                elif i == LAST:
                    ve.wait_ge(s_ll[3], 16)
                ve.tensor_add(a_[:, 2 * F : 3 * F], a_[:, F : 2 * F], x_[:, 3 * F : 4 * F]).then_inc(s_add)

            def emit_diff(i):
                # one overlapping-window sub: [d1 d2 d3] =
                # yt[:, F:4F] - yt[:, 0:3F]
                y_, o_ = yt[i % NY], ob[i % NO]
                if i >= NO:
                    ve.wait_ge(s_st1[i % NO], 16 * (i // NO))  # ob slot free
                ve.wait_ge(s_act, 2 * i + 2)  # Y0..Y3 ready
                ve.tensor_sub(o_[:, 0 : 3 * F], y_[:, F : 4 * F], y_[:, 0 : 3 * F]).then_inc(s_diff)

            def emit_diff_last():
                i = LAST
                y_, o_ = yt[i % NY], ob[i % NO]
                if i >= NO:
                    ve.wait_ge(s_st1[i % NO], 16 * (i // NO))
                for t in (1, 2, 3):
                    ve.wait_ge(s_act, 2 * i + 1 + t)  # Y_t ready
                    ve.tensor_sub(
                        o_[:, (t - 1) * F : t * F],
                        y_[:, t * F : (t + 1) * F],
                        y_[:, (t - 1) * F : t * F],
                    ).then_inc(s_diff)

            # adds run two chunks ahead of the diffs so ACT's silu2(i)
            # never waits on a just-emitted add
            emit_adds(0)
            emit_adds(1)
            for i in range(NCHUNK):
                if i + 2 < NCHUNK:
                    emit_adds(i + 2)
                if i == LAST:
                    emit_diff_last()
                else:
                    emit_diff(i)

        @block.scalar
        def _(se: bass.BassEngine):
            # Ramp: chunk-0 slices 2,3 and the chunk-1 load go out on
            # ACT's HWDGE ring, in parallel with SP's ramp DMAs (no
            # waits needed — all slots are empty at start).  After
            # these, ACT is pure compute: 2 silus per chunk into one
            # contiguous Y tile (Y0 from x0, Y1..Y3 from the sums).
            for t in (2, 3):
                se.dma_start(
                    out=xt[0][:, t * F : (t + 1) * F], in_=x_d[0][:, t]
                ).then_inc(s_l0[t], 16)
            k1, _u1 = lane_use[1]
            se.dma_start(out=xt[k1][:], in_=x_d[1]).then_inc(s_load[k1], 16)
            for i in range(NCHUNK):
                y_, a_ = yt[i % NY], at[i % NA]
                if i >= NY:
                    # yt slot free: t0 store + diff of chunk i-NY done
                    se.wait_ge(s_st0[i % NY], 16 * (i // NY))
                    se.wait_ge(s_diff, (i - NY) + 1)
                if i == 0:
                    se.wait_ge(s_l0[0], 16)
                elif i == LAST:
                    se.wait_ge(s_ll[0], 16)
                else:
                    k, use = lane_use[i]
                    se.wait_ge(s_load[k], 16 * use)
                se.activation(y_[:, 0:F], xt[i % NX][:, 0:F], act_fn).then_inc(s_act)
                if i < LAST:
                    se.wait_ge(s_add, 3 * i + 3)
                    se.activation(y_[:, F : 4 * F], a_[:, 0 : 3 * F], act_fn).then_inc(s_act)
                else:
                    # per-slice silus so each output slice can leave as
                    # soon as it's ready, shrinking the tail
                    for t in (1, 2, 3):
                        se.wait_ge(s_add, 3 * i + t)
                        se.activation(
                            y_[:, t * F : (t + 1) * F],
                            a_[:, (t - 1) * F : t * F],
                            act_fn,
                        ).then_inc(s_act)

    return nc


def get_nc(use_silu: bool = True):
    key = ("nc", use_silu)
    if key not in _NC_CACHE:
        _NC_CACHE[key] = _build_nc(use_silu)
    return _NC_CACHE[key]


def kernel(x: np.ndarray) -> np.ndarray:
    global LAST_RESULT
    from concourse.bass_utils import run_bass_kernel_spmd

    nc = get_nc()
    # fp16 on the wire: cast once on the host, then repack each core's
    # shard to the chunk-major [NCHUNK, P, T, F] DRAM layout the kernel
    # uses (contiguous per-partition DMA runs)
    x = np.asarray(x, dtype=np.float32).astype(np.float16)
    in_maps = [
        {"x": np.ascontiguousarray(
            x[:, :, c * LS : (c + 1) * LS, :]
            .reshape(T, NCHUNK, P, F)
            .transpose(1, 2, 0, 3)
        )}
        for c in range(NCORES)
    ]
    try:
        res = run_bass_kernel_spmd(
            nc, in_maps, list(range(NCORES)), trace=TRACE, tmpdir=TMPDIR,
            trace_cores=TRACE_CORES,
        )
    except Exception:
        # rare transient NRT_EXEC_UNIT_UNRECOVERABLE; the device recovers
        # on the next execution
        res = run_bass_kernel_spmd(
            nc, in_maps, list(range(NCORES)), trace=TRACE, tmpdir=TMPDIR,
            trace_cores=TRACE_CORES,
        )
    LAST_RESULT = res
    outs = [
        np.asarray(res.results[c]["out"], dtype=np.float32)
        .transpose(2, 0, 1, 3)
        .reshape(T, B, LS, D)
        for c in range(NCORES)
    ]
    return np.concatenate(outs, axis=2)


# revision 31
# speedup vs baseline: 1.1731x; 1.1731x over previous
"""Trainium2 Bass kernel for: out_t = silu(cumsum_t(x)) diff along T.

Reference (T, B, L, D) = (4, 2, 2048, 4096) f32:
    Y = silu(cumsum(x, axis=0)); out = concat([Y[:1], Y[1:] - Y[:-1]])

Strategy: shard L across the 8 NeuronCores (embarrassingly parallel; the
scan is over T=4 only).  Per core a raw-Bass 3-engine pipeline streams
16 chunks of [128 part x (4x1024)] fp16 through SBUF:

  SP  : ALL DMA — chunk loads plus the two output stores per chunk —
        on the SP HWDGE ring.  Every dma_start is issue-gated by a
        sequencer wait, so the ring never holds a not-ready transfer.
        The first NX loads go out as one unconditional burst (plus two
        chunk-0 slices and chunk 1 on ACT's ring, so both HWDGE units
        expand descriptors during the ramp); store(j) is then emitted
        at iteration j+NX, which makes it issue exactly when compute
        finishes chunk j — the ring stays saturated through fill,
        steady state, and drain.  First/last chunk loads are split per
        t-slice (faster ramp / shorter tail).
  DVE : running sums X1..X3 (3 fp16 tensor_adds into `at`), emitted two
        chunks ahead of the diffs so ACT never chases them, plus two
        tensor_subs per chunk: d1 = Y1 - Y0 [FD=F] and the overlapping-
        window d23 = yt[:, F:3F] - yt[:, 0:2F] [FD=2F] with
        yt = [Y1 Y2 Y3] contiguous.  All-16-bit operands keep every op
        in the 2x_1P perf mode (58 + FD/2 cycles, not 151 + FD).
  ACT : pure compute — 2 silu ACTIVATEs per chunk: silu(x0) [FD=F]
        straight into the out tile's t0 slot and silu([X1,X2,X3])
        [FD=3F] into `yt`.

The out tile ob = [Y0, d1, d2, d3] is contiguous (silu1 writes Y0
straight into its t0 slot), so ONE 8 KiB-per-partition store covers all
four t-slices — split 2/6 KiB stores measured ~15% lower SDMA
throughput than 8 KiB descriptors.

Explicit semaphores; every dma_start carries zero attached waits (the
DMA ISA encoding only fits one) — cross-engine deps are standalone
sequencer wait_ge instructions.  Same-engine RAW chains (the running
sums) are fenced with drain-backed waits on the engine's own semaphore.

The whole pipeline is fp16: the host casts x to fp16 before upload and
widens the fp16 output back to f32 (~6.5e-4 l2 rel err, well inside the
2e-2 gate).  HBM traffic is 32 MiB per core; with the ring saturated
end-to-end the SDMA engines run it in ~84-94 us, plus ~9 us of fixed
NEFF preamble (startup barrier + engine program loads).  Measured
93-111 us run-to-run (the spread is HBM arbitration between the 8
cores: 8 x 32 MiB / 2.9 TB/s chip bandwidth = 88 us is the chip-level
floor, so a core's time depends on how fairly the early all-read phase
is arbitrated), vs ~157 us for the f32-compute/bf16-out predecessor.
Compute queues sit under the DMA roofline (ACT ~65 us, DVE ~62 us
busy); engine arithmetic internals stay f32 (DVE/ACT compute in fp32
and round on write).
"""

import sys

if "/opt/trn_rl_repo" not in sys.path:
    sys.path.insert(0, "/opt/trn_rl_repo")

import numpy as np

T, B, L, D = 4, 2, 2048, 4096
NCORES = 8
LS = L // NCORES            # 256 rows of L per core
NPOS = B * LS * D           # 2_097_152 elements per t-slice per core
P = 128                     # SBUF partitions
F = 1024                    # free-dim elements per t-slice per chunk
NCHUNK = NPOS // (P * F)    # 16 chunk iterations per core
NX = 8                      # xt (input) slot count
NA = 4                      # at (running-sum) slot count (adds run 2 ahead)
NY = 7                      # yt (silu) slot count
NO = 8                      # ob (diff) slot count
# NO is deeper than strictly needed for overlap: a chunk's silu1 waits
# on store(i-NO) COMPLETING, and that store sits behind the ring's FIFO
# backlog — shallow slots make back-half compute ring-gated.
# Stores trail loads by NX chunks on the SP ring: store(j) is emitted at
# SP iteration j+NX, whose load slot-wait (adds(j) done) is satisfied at
# the same moment as the store's own diff(j) wait — i.e. each store is
# issued exactly when compute finishes its chunk, and the first NX loads
# go out as one unconditional burst that keeps the ring fed through the
# pipeline fill.

_NC_CACHE = {}
LAST_RESULT = None
TRACE = False
TRACE_CORES = None
TMPDIR = None


def _build_nc(use_silu: bool = True):
    import concourse.bass as bass
    from concourse import mybir

    f16 = mybir.dt.float16
    act_fn = (
        mybir.ActivationFunctionType.Silu
        if use_silu
        else mybir.ActivationFunctionType.Sigmoid
    )

    nc = bass.Bass("TRN2", debug=False)
    # Chunk-major DRAM layout [NCHUNK, P, T, F] (host repacks): each
    # partition's chunk data is one contiguous 8 KiB run, so every DMA
    # is a straight copy with maximal descriptors.
    x_d = nc.declare_dram_parameter("x", [NCHUNK, P, T, F], f16, isOutput=False)
    o_d = nc.declare_dram_parameter("out", [NCHUNK, P, T, F], f16, isOutput=True)

    TF = T * F
    # Flat free dims so every engine AP is a single contiguous run
    # (keeps the DVE perf-mode detection trivially satisfied).
    # ob = [Y0, d1, d2, d3] is the full output chunk: one 8 KiB-per-
    # partition store per chunk (2/6 KiB split stores measured ~15%
    # lower SDMA throughput than 8 KiB descriptors).
    xt = [nc.alloc_sbuf_tensor(f"xt{s}", [P, TF], f16).ap() for s in range(NX)]
    at = [nc.alloc_sbuf_tensor(f"at{s}", [P, 3 * F], f16).ap() for s in range(NA)]
    yt = [nc.alloc_sbuf_tensor(f"yt{s}", [P, 3 * F], f16).ap() for s in range(NY)]
    ob = [nc.alloc_sbuf_tensor(f"ob{s}", [P, TF], f16).ap() for s in range(NO)]

    LAST = NCHUNK - 1

    # Regular-chunk load lanes: chunk 0 and LAST use dedicated split
    # per-slice sems; chunks 1..LAST-1 rotate over NX lanes.  A lane's
    # next DMA never overlaps its previous one (slot-reuse waits
    # guarantee it), so ">= 16*n" thresholds stay sound.
    lane_use = {}
    _cnt = [0] * NX
    for i in range(1, LAST):
        k = i % NX
        _cnt[k] += 1
        lane_use[i] = (k, _cnt[k])

    # Semaphore landmarks:
    #   s_add : add_t(i)  -> 3i+t  (t = 1..3)
    #   s_act : regular chunk i: silu1 -> 2i+1, silu2 -> 2i+2;
    #           LAST: silu1 -> 2L+1, then per-slice silu_t -> 2L+1+t
    #   s_diff: regular chunk i: d1 -> 2i+1, d23 -> 2i+2;
    #           LAST per-slice d_t -> 2*LAST + t
    # Store lanes: s_st rotates over the NO ob slots; LAST uses s_ls.

    import contextlib

    with contextlib.ExitStack() as es:
        block = es.enter_context(nc.Block())
        s_load = [es.enter_context(nc.semaphore(f"s_load{k}")) for k in range(NX)]
        s_st = [es.enter_context(nc.semaphore(f"s_st{k}")) for k in range(NO)]
        s_add = es.enter_context(nc.semaphore("s_add"))
        s_act = es.enter_context(nc.semaphore("s_act"))
        s_diff = es.enter_context(nc.semaphore("s_diff"))
        s_l0 = [es.enter_context(nc.semaphore(f"s_l0_{t}")) for t in range(T)]
        s_ll = [es.enter_context(nc.semaphore(f"s_ll{t}")) for t in range(T)]
        s_ls = [es.enter_context(nc.semaphore(f"s_ls{t}")) for t in range(T)]

        @block.sync
        def _(sp: bass.BassEngine):
            def emit_load(i):
                j = i - NX if i >= NX else -1
                if j >= 0:
                    # xt slot free: DVE adds + ACT silu1 of chunk j done
                    # reading it.  (Transitively covers load j's
                    # completion, so this lane's previous inc is
                    # observed before re-use.)
                    sp.wait_ge(s_add, 3 * j + 3)
                    sp.wait_ge(s_act, 2 * j + 1)
                if i == 0:
                    # split: smaller first DMAs ramp the SDMA engines
                    # sooner and let compute start per slice.  Slices
                    # 2,3 (and chunk 1) are issued from ACT's ring so
                    # both HWDGE units expand descriptors concurrently
                    # during the ramp.
                    for t in range(2):
                        sp.dma_start(
                            out=xt[0][:, t * F : (t + 1) * F], in_=x_d[0][:, t]
                        ).then_inc(s_l0[t], 16)
                elif i == 1:
                    return  # issued from ACT (ramp)
                elif i == LAST:
                    for t in range(T):
                        sp.dma_start(
                            out=xt[i % NX][:, t * F : (t + 1) * F], in_=x_d[i][:, t]
                        ).then_inc(s_ll[t], 16)
                else:
                    k, _use = lane_use[i]
                    sp.dma_start(out=xt[k][:], in_=x_d[i]).then_inc(s_load[k], 16)

            def emit_store(j):
                # one 8 KiB/partition store: ob = [Y0, d1, d2, d3]
                sp.wait_ge(s_diff, 2 * j + 2)  # d1+d23 done (covers silu1 too)
                if j >= NO:
                    # observe this lane's previous inc before re-inc'ing
                    sp.wait_ge(s_st[j % NO], 16 * (j // NO))
                sp.dma_start(out=o_d[j], in_=ob[j % NO][:]).then_inc(
                    s_st[j % NO], 16
                )

            for i in range(NCHUNK):
                emit_load(i)
                if i - NX >= 0 and i - NX < LAST:
                    emit_store(i - NX)
            for j in range(max(NCHUNK - NX, 0), LAST):
                emit_store(j)
            # last chunk: per-slice stores as each slice becomes ready
            o_ = ob[LAST % NO]
            sp.wait_ge(s_act, 2 * LAST + 1)  # silu1(LAST) drained (Y0)
            sp.dma_start(out=o_d[LAST][:, 0], in_=o_[:, 0:F]).then_inc(s_ls[0], 16)
            for t in (1, 2, 3):
                sp.wait_ge(s_diff, 2 * LAST + t)
                sp.dma_start(
                    out=o_d[LAST][:, t], in_=o_[:, t * F : (t + 1) * F]
                ).then_inc(s_ls[t], 16)
            # drain: HWDGE DMAs on one ring complete in FIFO order, and
            # every store precedes the last chunk's slice stores on the
            # SP ring — waiting the four s_ls sems covers them all
            for t in range(T):
                sp.wait_ge(s_ls[t], 16)

        @block.vector
        def _(ve: bass.BassEngine):
            def emit_adds(i):
                x_, a_ = xt[i % NX], at[i % NA]
                if i >= NA:
                    # at slot free: silu2 of chunk i-NA done reading it
                    ve.wait_ge(s_act, 2 * (i - NA) + 2)
                if i == 0:
                    ve.wait_ge(s_l0[0], 16)
                    ve.wait_ge(s_l0[1], 16)
                elif i == LAST:
                    ve.wait_ge(s_ll[0], 16)
                    ve.wait_ge(s_ll[1], 16)
                else:
                    k, use = lane_use[i]
                    ve.wait_ge(s_load[k], 16 * use)
                ve.tensor_add(a_[:, 0:F], x_[:, 0:F], x_[:, F : 2 * F]).then_inc(s_add)
                # same-engine RAW needs a drain-backed sem wait
                ve.wait_ge(s_add, 3 * i + 1)
                if i == 0:
                    ve.wait_ge(s_l0[2], 16)
                elif i == LAST:
                    ve.wait_ge(s_ll[2], 16)
                ve.tensor_add(a_[:, F : 2 * F], a_[:, 0:F], x_[:, 2 * F : 3 * F]).then_inc(s_add)
                ve.wait_ge(s_add, 3 * i + 2)
                if i == 0:
                    ve.wait_ge(s_l0[3], 16)
                elif i == LAST:
                    ve.wait_ge(s_ll[3], 16)
                ve.tensor_add(a_[:, 2 * F : 3 * F], a_[:, F : 2 * F], x_[:, 3 * F : 4 * F]).then_inc(s_add)

            def emit_diff(i):
                # d1 = Y1 - Y0 (Y0 lives in the out tile's t0 slot);
                # d23 = one overlapping-window sub inside yt = [Y1 Y2 Y3]
                y_, o_ = yt[i % NY], ob[i % NO]
                if i >= NO:
                    ve.wait_ge(s_st[i % NO], 16 * (i // NO))  # ob slot free
                ve.wait_ge(s_act, 2 * i + 2)  # Y1..Y3 (and ob t0 = Y0) ready
                ve.tensor_sub(o_[:, F : 2 * F], y_[:, 0:F], o_[:, 0:F]).then_inc(s_diff)
                ve.tensor_sub(o_[:, 2 * F : 4 * F], y_[:, F : 3 * F], y_[:, 0 : 2 * F]).then_inc(s_diff)

            def emit_diff_last():
                i = LAST
                y_, o_ = yt[i % NY], ob[i % NO]
                if i >= NO:
                    ve.wait_ge(s_st[i % NO], 16 * (i // NO))
                ve.wait_ge(s_act, 2 * i + 2)  # Y1 ready
                ve.tensor_sub(o_[:, F : 2 * F], y_[:, 0:F], o_[:, 0:F]).then_inc(s_diff)
                for t in (2, 3):
                    ve.wait_ge(s_act, 2 * i + 1 + t)  # Y_t ready
                    ve.tensor_sub(
                        o_[:, t * F : (t + 1) * F],
                        y_[:, (t - 1) * F : t * F],
                        y_[:, (t - 2) * F : (t - 1) * F],
                    ).then_inc(s_diff)

            # adds run two chunks ahead of the diffs so ACT's silu2(i)
            # never waits on a just-emitted add
            emit_adds(0)
            emit_adds(1)
            for i in range(NCHUNK):
                if i + 2 < NCHUNK:
                    emit_adds(i + 2)
                if i == LAST:
                    emit_diff_last()
                else:
                    emit_diff(i)

        @block.scalar
        def _(se: bass.BassEngine):
            # Ramp: chunk-0 slices 2,3 and the chunk-1 load go out on
            # ACT's HWDGE ring, in parallel with SP's ramp DMAs (no
            # waits needed — all slots are empty at start).  After
            # these, ACT is pure compute: 2 silus per chunk into one
            # contiguous Y tile (Y0 from x0, Y1..Y3 from the sums).
            for t in (2, 3):
                se.dma_start(
                    out=xt[0][:, t * F : (t + 1) * F], in_=x_d[0][:, t]
                ).then_inc(s_l0[t], 16)
            k1, _u1 = lane_use[1]
            se.dma_start(out=xt[k1][:], in_=x_d[1]).then_inc(s_load[k1], 16)
            for i in range(NCHUNK):
                y_, a_, o_ = yt[i % NY], at[i % NA], ob[i % NO]
                if i >= NO:
                    # ob slot free: store of chunk i-NO done (silu1
                    # writes the slot's t0 before DVE's diffs fill it)
                    se.wait_ge(s_st[i % NO], 16 * (i // NO))
                if i == 0:
                    se.wait_ge(s_l0[0], 16)
                elif i == LAST:
                    se.wait_ge(s_ll[0], 16)
                else:
                    k, use = lane_use[i]
                    se.wait_ge(s_load[k], 16 * use)
                se.activation(o_[:, 0:F], xt[i % NX][:, 0:F], act_fn).then_inc(s_act)
                if i >= NY:
                    # yt slot free: d23 of chunk i-NY done reading it
                    se.wait_ge(s_diff, 2 * (i - NY) + 2)
                if i < LAST:
                    se.wait_ge(s_add, 3 * i + 3)
                    se.activation(y_[:, 0 : 3 * F], a_[:, 0 : 3 * F], act_fn).then_inc(s_act)
                else:
                    # per-slice silus so each output slice can leave as
                    # soon as it's ready, shrinking the tail
                    for t in (1, 2, 3):
                        se.wait_ge(s_add, 3 * i + t)
                        se.activation(
                            y_[:, (t - 1) * F : t * F],
                            a_[:, (t - 1) * F : t * F],
                            act_fn,
                        ).then_inc(s_act)

    return nc


def get_nc(use_silu: bool = True):
    key = ("nc", use_silu)
    if key not in _NC_CACHE:
        _NC_CACHE[key] = _build_nc(use_silu)
    return _NC_CACHE[key]


def kernel(x: np.ndarray) -> np.ndarray:
    global LAST_RESULT
    from concourse.bass_utils import run_bass_kernel_spmd

    nc = get_nc()
    # fp16 on the wire: cast once on the host, then repack each core's
    # shard to the chunk-major [NCHUNK, P, T, F] DRAM layout the kernel
    # uses (contiguous per-partition DMA runs)
    x = np.asarray(x, dtype=np.float32).astype(np.float16)
    in_maps = [
        {"x": np.ascontiguousarray(
            x[:, :, c * LS : (c + 1) * LS, :]
            .reshape(T, NCHUNK, P, F)
            .transpose(1, 2, 0, 3)
        )}
        for c in range(NCORES)
    ]
    try:
        res = run_bass_kernel_spmd(
            nc, in_maps, list(range(NCORES)), trace=TRACE, tmpdir=TMPDIR,
            trace_cores=TRACE_CORES,
        )
    except Exception:
        # rare transient NRT_EXEC_UNIT_UNRECOVERABLE; the device recovers
        # on the next execution
        res = run_bass_kernel_spmd(
            nc, in_maps, list(range(NCORES)), trace=TRACE, tmpdir=TMPDIR,
            trace_cores=TRACE_CORES,
        )
    LAST_RESULT = res
    outs = [
        np.asarray(res.results[c]["out"], dtype=np.float32)
        .transpose(2, 0, 1, 3)
        .reshape(T, B, LS, D)
        for c in range(NCORES)
    ]
    return np.concatenate(outs, axis=2)
